# revision 1
# baseline (speedup 1.0000x reference)
"""GAT-style fused GNN message passing kernel for Trainium2 (8 NeuronCores).

Computes (matching the reference):
    h      = x @ W
    e_ij   = leaky_relu(h_i . a[:D] + h_j . a[D:])   (i=row=dest, j=col=src)
    att    = segment_softmax(e, by row)              (|e| <= ~3 so exp() direct;
                                                      identical softmax value)
    h'     = segment_sum(att * h[col], by row);  out = elu(h')

Design: edges sorted by dest row on the host, cut into blocks of <=128 dest
rows / <=2048 edges; blocks distributed over 8 cores (uniform SPMD
structure: B blocks x 16 tiles x 128 edge slots per core, pads masked out).
Each core builds the full node table HAUG[n] = [h(128)|1.0|s_dst|pad2]
(132 f32 = 528 B) plus a node-major s_src vector, then per tile
indirect-DMA-gathers the 128 edge rows (by col), extracts per-edge s_src
via a mask-matrix reduce against a per-block broadcast of the contiguous
s_src row (the gather itself replicates it to all partitions), builds
SEL[e,d] = (d==dest_local[e]) * exp(lrelu(s_src+s_dst)) and accumulates
PSUM[d,0:129] += SEL.T @ [h | 1] on the TensorEngine (ones column = softmax
denominator Z).  Epilogue: elu(out * 1/max(Z,eps)).  No collectives.
"""

import numpy as np
from contextlib import ExitStack

P = 128
D = 128
ALPHA = 0.2
EPS = 1e-8
TCOLS = 132        # table row: h(0:128) | ones(128) | s_dst(129) | pad(130:132)
RHS_COLS = 129
SDST_COL = 129
PAD_DEST = 1.0e9
N_CORES = 8

_PROG_CACHE = {}


# ---------------------------------------------------------------- host prep
def _build_blocks(row, col, n_nodes, epb):
    E = row.shape[0]
    order = np.argsort(row, kind="stable")
    rs = row[order].astype(np.int64)
    cs = col[order].astype(np.int64)
    deg = np.bincount(row, minlength=n_nodes).astype(np.int64)
    assert deg.max() <= epb, "single row exceeds block capacity"
    cum = np.concatenate([[0], np.cumsum(deg)])

    row_start, n_rows, e_start, e_cnt = [], [], [], []
    r = 0
    while r < n_nodes:
        r2 = min(r + P, n_nodes)
        hi = int(np.searchsorted(cum, cum[r] + epb, side="right")) - 1
        r2 = min(r2, max(hi, r + 1))
        row_start.append(r)
        n_rows.append(r2 - r)
        e_start.append(int(cum[r]))
        e_cnt.append(int(cum[r2] - cum[r]))
        r = r2
    return rs, cs, (
        np.array(row_start, np.int64),
        np.array(n_rows, np.int64),
        np.array(e_start, np.int64),
        np.array(e_cnt, np.int64),
    )


def _prep_host(row, col, n_nodes, t_tiles):
    epb = t_tiles * P
    nblk = (n_nodes + P - 1) // P
    rs, cs, (row_start, n_rows, e_start, e_cnt) = _build_blocks(
        row, col, n_nodes, epb
    )
    E = rs.shape[0]
    BT = len(row_start)
    B = -(-BT // N_CORES)
    B = -(-B // 8) * 8
    BTP = B * N_CORES

    def padB(a, fill):
        out = np.full(BTP, fill, a.dtype)
        out[:BT] = a
        return out

    row_start_p = padB(row_start, 0)
    e_start_p = padB(e_start, 0)
    e_cnt_p = padB(e_cnt, 0)

    slot = np.arange(epb)
    idxmat = e_start_p[:, None] + slot[None, :]
    maskm = slot[None, :] < e_cnt_p[:, None]
    idxc = np.minimum(idxmat, max(E - 1, 0))
    colm = np.where(maskm, cs[idxc], 0).astype(np.int32)
    rowm = rs[idxc]
    destm = np.where(
        maskm, (rowm - row_start_p[:, None]).astype(np.float32), PAD_DEST
    ).astype(np.float32)

    def to_core_layout(a):
        a = a.reshape(BTP, t_tiles, P).transpose(0, 2, 1)      # [BTP, 128, T]
        a = a.reshape(N_CORES, B // 8, 8, P, t_tiles)
        a = a.transpose(0, 1, 3, 2, 4)                         # [c, G8, p, 8, T]
        return np.ascontiguousarray(a.reshape(N_CORES, B // 8, P, 8 * t_tiles))

    # per-block s_src gather offsets: row_start replicated to 128 partitions
    ub = np.broadcast_to(
        row_start_p.astype(np.int32)[:, None], (BTP, P)
    )                                                          # [BTP, p]
    ub = ub.reshape(N_CORES, B // 8, 8, P).transpose(0, 1, 3, 2)
    ub = np.ascontiguousarray(ub)                              # [c, G8, 128, 8]

    return {
        "col": to_core_layout(colm),
        "dest": to_core_layout(destm),
        "ubofs": ub,
        "B": B,
        "BT": BT,
        "row_start": row_start,
        "n_rows": n_rows,
        "nblk": nblk,
    }


# ---------------------------------------------------------------- device code
def _build_program(n_nodes, B, t_tiles, nblk):
    import os as _os
    _skip_h = _os.environ.get("GAT_SKIP_H") == "1"
    _skip_agg = _os.environ.get("GAT_SKIP_AGG") == "1"
    _skip_srow = _os.environ.get("GAT_SKIP_SROW") == "1"
    import concourse.bass as bass
    import concourse.tile as tile
    from concourse import bacc, mybir

    fp32 = mybir.dt.float32
    i32 = mybir.dt.int32
    AF = mybir.ActivationFunctionType
    OP = mybir.AluOpType
    AX = mybir.AxisListType

    n_pad = nblk * P
    G8 = B // 8

    nc = bacc.Bacc(
        "TRN2", target_bir_lowering=False, debug=False, num_devices=N_CORES
    )

    xT = nc.dram_tensor("xT", [P, n_pad], fp32, kind="ExternalInput").ap()
    W_in = nc.dram_tensor("W", [P, P], fp32, kind="ExternalInput").ap()
    WT_in = nc.dram_tensor("WT", [P, P], fp32, kind="ExternalInput").ap()
    a2_in = nc.dram_tensor("a2", [P, 2], fp32, kind="ExternalInput").ap()
    iota_in = nc.dram_tensor("iota", [P, P], fp32, kind="ExternalInput").ap()
    colidx = nc.dram_tensor(
        "colidx", [G8, P, 8 * t_tiles], i32, kind="ExternalInput"
    ).ap()
    destin = nc.dram_tensor(
        "destin", [G8, P, 8 * t_tiles], fp32, kind="ExternalInput"
    ).ap()
    ubofs = nc.dram_tensor("ubofs", [G8, P, 8], i32, kind="ExternalInput").ap()
    out_c = nc.dram_tensor("out", [B * P, P], fp32, kind="ExternalOutput").ap()

    haug = nc.dram_tensor("haug", [n_pad, TCOLS], fp32).ap()
    stab = nc.dram_tensor("stab", [n_pad + P, 1], fp32).ap()

    with tile.TileContext(nc) as tc:
        with ExitStack() as ctx:
            cpool = ctx.enter_context(tc.tile_pool(name="const", bufs=1))
            W_sb = cpool.tile([P, P], fp32)
            nc.sync.dma_start(W_sb[:], W_in[:])
            WT_sb = cpool.tile([P, P], fp32)
            nc.sync.dma_start(WT_sb[:], WT_in[:])
            a_sb = cpool.tile([P, 2], fp32)
            nc.sync.dma_start(a_sb[:], a2_in[:])
            iota_sb = cpool.tile([P, P], fp32)
            nc.sync.dma_start(iota_sb[:], iota_in[:])

            wa_sb = cpool.tile([P, 2], fp32)
            with tc.tile_pool(name="wapsum", bufs=1, space="PSUM") as wap:
                wa_ps = wap.tile([P, 2], fp32)
                nc.tensor.matmul(
                    out=wa_ps[:], lhsT=WT_sb[:], rhs=a_sb[:], start=True, stop=True
                )
                nc.vector.tensor_copy(wa_sb[:], wa_ps[:])

            # ---------------- phase H ----------------
            GH = 4
            n_groups = 0 if _skip_h else -(-nblk // GH)
            with ExitStack() as hctx:
                xpool = hctx.enter_context(tc.tile_pool(name="xchunk", bufs=3))
                hpsum = hctx.enter_context(
                    tc.tile_pool(name="hpsum", bufs=2, space="PSUM")
                )
                spsum = hctx.enter_context(
                    tc.tile_pool(name="spsum", bufs=2, space="PSUM")
                )
                rpsum = hctx.enter_context(
                    tc.tile_pool(name="rpsum", bufs=2, space="PSUM")
                )
                stgp = hctx.enter_context(tc.tile_pool(name="haugstg", bufs=3))
                srp = hctx.enter_context(tc.tile_pool(name="srow", bufs=3))

                for g in range(n_groups):
                    b0 = g * GH
                    gsz = min(GH, nblk - b0)
                    chunk = xpool.tile([P, GH * P], fp32, tag="xchunk")
                    nc.sync.dma_start(
                        chunk[:, : gsz * P], xT[:, b0 * P : (b0 + gsz) * P]
                    )
                    ph = hpsum.tile([P, GH * P], fp32, tag="hps")
                    ps = spsum.tile([P, GH * 2], fp32, tag="sps")
                    pr = rpsum.tile([P, GH * P], fp32, tag="rps")
                    for u in range(gsz):
                        lhs = chunk[:, u * P : (u + 1) * P]
                        nc.tensor.matmul(
                            out=ph[:, u * P : (u + 1) * P],
                            lhsT=lhs,
                            rhs=W_sb[:],
                            start=True,
                            stop=True,
                        )
                        nc.tensor.matmul(
                            out=ps[:, u * 2 : (u + 1) * 2],
                            lhsT=lhs,
                            rhs=wa_sb[:],
                            start=True,
                            stop=True,
                        )
                    # s_src row (node-major): [1, gsz*128]
                    if not _skip_srow:
                      nc.tensor.matmul(
                        out=pr[0:2, : gsz * P],
                        lhsT=wa_sb[:],
                        rhs=chunk[:, : gsz * P],
                        start=True,
                        stop=True,
                      )
                      srow = srp.tile([1, GH * P], fp32, tag="srow")
                      nc.vector.tensor_copy(srow[:, : gsz * P], pr[0:1, : gsz * P])
                      import os as _os2
                      if _os2.environ.get("GAT_SKIP_SDMA") != "1":
                          nc.sync.dma_start(
                              bass.AP(stab.tensor, b0 * P, [[1, 1], [1, gsz * P]]),
                              srow[:, : gsz * P],
                          )

                    stg = stgp.tile([P, GH * TCOLS], fp32, tag="stg")
                    dst_h = bass.AP(
                        stg.tensor, stg[:].offset, [stg[:].ap[0], [TCOLS, gsz], [1, P]]
                    )
                    src_h = bass.AP(
                        ph.tensor, ph[:].offset, [ph[:].ap[0], [P, gsz], [1, P]]
                    )
                    if g % 2 == 0:
                        nc.vector.tensor_copy(dst_h, src_h)
                    else:
                        nc.scalar.copy(dst_h, src_h)
                    dst_one = bass.AP(
                        stg.tensor,
                        stg[:].offset + 128,
                        [stg[:].ap[0], [TCOLS, gsz], [1, 1]],
                    )
                    nc.vector.memset(dst_one, 1.0)
                    dst_sd = bass.AP(
                        stg.tensor,
                        stg[:].offset + SDST_COL,
                        [stg[:].ap[0], [TCOLS, gsz], [1, 1]],
                    )
                    src_sd = bass.AP(
                        ps.tensor, ps[:].offset + 1, [ps[:].ap[0], [2, gsz], [1, 1]]
                    )
                    nc.vector.tensor_copy(dst_sd, src_sd)
                    dst_pad = bass.AP(
                        stg.tensor,
                        stg[:].offset + 130,
                        [stg[:].ap[0], [TCOLS, gsz], [1, 2]],
                    )
                    nc.vector.memset(dst_pad, 0.0)
                    dst_dram = bass.AP(
                        haug.tensor,
                        b0 * P * TCOLS,
                        [[TCOLS, P], [P * TCOLS, gsz], [1, TCOLS]],
                    )
                    nc.sync.dma_start(dst_dram, stg[:, : gsz * TCOLS])

                # zero the stab tail pad (avoid NaN garbage)
                zt = srp.tile([1, P], fp32, tag="ztail")
                nc.vector.memset(zt[:], 0.0)
                nc.sync.dma_start(
                    bass.AP(stab.tensor, n_pad, [[1, 1], [1, P]]), zt[:]
                )

            # ---------------- phase AGG ----------------
            with ExitStack() as actx:
                ldp = actx.enter_context(tc.tile_pool(name="edgeld", bufs=2))
                gpool = actx.enter_context(tc.tile_pool(name="gather", bufs=3))
                ubp = actx.enter_context(tc.tile_pool(name="ub", bufs=3))
                mp = actx.enter_context(tc.tile_pool(name="masks", bufs=2))
                tmpp = actx.enter_context(tc.tile_pool(name="tmp", bufs=3))
                exp_p = actx.enter_context(tc.tile_pool(name="expe", bufs=3))
                selp = actx.enter_context(tc.tile_pool(name="sel", bufs=4))
                opsum = actx.enter_context(
                    tc.tile_pool(name="outpsum", bufs=4, space="PSUM")
                )
                epip = actx.enter_context(tc.tile_pool(name="epi", bufs=2))
                outp = actx.enter_context(tc.tile_pool(name="outstg", bufs=2))

                for g8 in range(0 if _skip_agg else G8):
                    colT = ldp.tile([P, 8 * t_tiles], i32, tag="colT")
                    nc.sync.dma_start(colT[:], colidx[g8])
                    destT = ldp.tile([P, 8 * t_tiles], fp32, tag="destT")
                    nc.sync.dma_start(destT[:], destin[g8])
                    ubT = ldp.tile([P, 8], i32, tag="ubT")
                    nc.sync.dma_start(ubT[:], ubofs[g8])

                    for b_in in range(8):
                        # U_B[p, :] = s_src[row_start : row_start+128] (replicated)
                        UB = ubp.tile([P, P], fp32, tag="UB")
                        nc.gpsimd.indirect_dma_start(
                            out=UB[:],
                            out_offset=None,
                            in_=stab[:, :],
                            in_offset=bass.IndirectOffsetOnAxis(
                                ap=ubT[:, b_in : b_in + 1], axis=0
                            ),
                        )
                        G = gpool.tile([P, t_tiles * TCOLS], fp32, tag="G")
                        M_all = mp.tile([P, t_tiles * P], fp32, tag="M")
                        ust = exp_p.tile([P, t_tiles], fp32, tag="ust")
                        for u in range(t_tiles):
                            tu = b_in * t_tiles + u
                            nc.gpsimd.indirect_dma_start(
                                out=G[:, u * TCOLS : (u + 1) * TCOLS],
                                out_offset=None,
                                in_=haug[:, :],
                                in_offset=bass.IndirectOffsetOnAxis(
                                    ap=colT[:, tu : tu + 1], axis=0
                                ),
                            )
                            M = M_all[:, u * P : (u + 1) * P]
                            nc.vector.tensor_scalar(
                                M,
                                iota_sb[:],
                                destT[:, tu : tu + 1],
                                None,
                                OP.is_equal,
                            )
                            tmp = tmpp.tile([P, P], fp32, tag="tmp")
                            nc.vector.tensor_tensor(tmp[:], UB[:], M, OP.mult)
                            nc.vector.tensor_reduce(
                                ust[:, u : u + 1], tmp[:], AX.X, OP.add
                            )
                        # e = s_src + s_dst ; lrelu ; exp     [128, T]
                        sdst_v = bass.AP(
                            G.tensor,
                            G[:].offset + SDST_COL,
                            [G[:].ap[0], [TCOLS, t_tiles], [1, 1]],
                        )
                        e_t = exp_p.tile([P, t_tiles], fp32, tag="e_t")
                        nc.vector.tensor_tensor(e_t[:], ust[:], sdst_v, OP.add)
                        e_a = exp_p.tile([P, t_tiles], fp32, tag="e_a")
                        nc.vector.tensor_scalar_mul(e_a[:], e_t[:], ALPHA)
                        lr = exp_p.tile([P, t_tiles], fp32, tag="lr")
                        nc.vector.tensor_tensor(lr[:], e_t[:], e_a[:], OP.max)
                        expe = exp_p.tile([P, t_tiles], fp32, tag="expe")
                        nc.scalar.activation(expe[:], lr[:], AF.Exp)

                        po = opsum.tile([P, RHS_COLS], fp32, tag="po")
                        for u in range(t_tiles):
                            sel = selp.tile([P, P], fp32, tag="sel")
                            nc.vector.tensor_scalar(
                                sel[:],
                                M_all[:, u * P : (u + 1) * P],
                                expe[:, u : u + 1],
                                None,
                                OP.mult,
                            )
                            nc.tensor.matmul(
                                out=po[:],
                                lhsT=sel[:],
                                rhs=G[:, u * TCOLS : u * TCOLS + RHS_COLS],
                                start=(u == 0),
                                stop=(u == t_tiles - 1),
                            )

                        zc = epip.tile([P, 1], fp32, tag="zc")
                        nc.vector.tensor_scalar_max(zc[:], po[:, 128:129], EPS)
                        rz = epip.tile([P, 1], fp32, tag="rz")
                        nc.vector.reciprocal(rz[:], zc[:])
                        neg = epip.tile([P, P], fp32, tag="neg")
                        nc.vector.tensor_scalar(
                            neg[:], po[:, :P], rz[:], 0.0, OP.mult, OP.min
                        )
                        pos = epip.tile([P, P], fp32, tag="pos")
                        nc.vector.tensor_scalar(
                            pos[:], po[:, :P], rz[:], 0.0, OP.mult, OP.max
                        )
                        eneg = epip.tile([P, P], fp32, tag="eneg")
                        nc.scalar.activation(eneg[:], neg[:], AF.Exp)
                        if b_in % 4 == 0:
                            out4 = outp.tile([P, 4 * P], fp32, tag="out4")
                        osl = out4[:, (b_in % 4) * P : (b_in % 4 + 1) * P]
                        nc.vector.tensor_tensor(osl, eneg[:], pos[:], OP.add)
                        nc.vector.tensor_scalar_sub(osl, osl, 1.0)
                        if b_in % 4 == 3:
                            b = g8 * 8 + b_in
                            dst = bass.AP(
                                out_c.tensor,
                                (b - 3) * P * P,
                                [[P, P], [P * P, 4], [1, P]],
                            )
                            nc.sync.dma_start(dst, out4[:])

    nc.compile()
    return nc


# ---------------------------------------------------------------- entry point
def _get_program(n_nodes, B, t_tiles, nblk):
    key = (n_nodes, B, t_tiles, nblk)
    if key not in _PROG_CACHE:
        _PROG_CACHE[key] = _build_program(n_nodes, B, t_tiles, nblk)
    return _PROG_CACHE[key]


def kernel(x, row, col, W, a, t_tiles=16, trace=False):
    from concourse.bass_utils import run_bass_kernel_spmd

    n_nodes = x.shape[0]
    prep = _prep_host(np.asarray(row), np.asarray(col), n_nodes, t_tiles)
    B, BT, nblk = prep["B"], prep["BT"], prep["nblk"]
    n_pad = nblk * P

    nc = _get_program(n_nodes, B, t_tiles, nblk)

    x = np.asarray(x, np.float32)
    W = np.asarray(W, np.float32)
    a = np.asarray(a, np.float32).reshape(2 * D)
    xT = np.zeros((P, n_pad), np.float32)
    xT[:, :n_nodes] = x.T
    a2 = np.ascontiguousarray(np.stack([a[:D], a[D:]], axis=1))
    WT = np.ascontiguousarray(W.T)
    iota = np.broadcast_to(np.arange(P, dtype=np.float32), (P, P)).copy()

    in_maps = []
    for c in range(N_CORES):
        in_maps.append(
            {
                "xT": xT,
                "W": W,
                "WT": WT,
                "a2": a2,
                "iota": iota,
                "colidx": prep["col"][c],
                "destin": prep["dest"][c],
                "ubofs": prep["ubofs"][c],
            }
        )

    res = run_bass_kernel_spmd(nc, in_maps, list(range(N_CORES)), trace=trace)
    global LAST_EXEC_NS
    LAST_EXEC_NS = res.exec_time_ns

    out = np.zeros((n_nodes, D), np.float32)
    row_start, n_rows = prep["row_start"], prep["n_rows"]
    for g in range(BT):
        c, b = g // B, g % B
        r0, nr = int(row_start[g]), int(n_rows[g])
        out[r0 : r0 + nr] = res.results[c]["out"][b * P : b * P + nr]
    return out


LAST_EXEC_NS = None

